# revision 3
# baseline (speedup 1.0000x reference)
"""BGNN4VD (bidirectional GAT + CNN head) on 8 Trainium2 NeuronCores.

Strategy: the model's dense GEMMs (embed, 12 GAT projections + attention
vectors, 6 combine layers) are executed on-device via a single generic
row-sharded GEMM kernel (rows split 8 ways, weights replicated, N padded
to 512). The data-dependent edge softmax/aggregation and the tiny CNN
head run host-side between launches.
"""
import os
import numpy as np

N_NODES = 100000
N_GRAPHS = 64
HID = 256
HALF = 128
N_LAYERS = 6
N_CORES = 8
ROWS_PAD = 100352          # 8 * 12544, multiple of 128*8
ROWS_SH = ROWS_PAD // N_CORES   # 12544 rows per core
KDIM = 256                 # padded contraction dim
NDIM = 512                 # padded output dim
TILES = ROWS_SH // 128     # 98

_cache = {}


def _build_gemm(n_out):
    """y[12544,n_out] = blend(xT.T @ W + b) per core; blend = max(z, z*s)."""
    from concourse import mybir, bacc
    import concourse.tile as tile

    nc = bacc.Bacc("TRN2", target_bir_lowering=False, debug=False,
                   num_devices=N_CORES)
    f32 = mybir.dt.float32
    xT = nc.dram_tensor("xT", [KDIM, ROWS_SH], f32, kind="ExternalInput")
    w = nc.dram_tensor("w", [KDIM, n_out], f32, kind="ExternalInput")
    b = nc.dram_tensor("b", [1, n_out], f32, kind="ExternalInput")
    s = nc.dram_tensor("s", [128, 1], f32, kind="ExternalInput")
    y = nc.dram_tensor("y", [ROWS_SH, n_out], f32, kind="ExternalOutput")

    with tile.TileContext(nc) as tc:
        with tc.tile_pool(name="const", bufs=1) as cpool, \
             tc.tile_pool(name="io", bufs=4) as iop, \
             tc.tile_pool(name="ps", bufs=4, space="PSUM") as psp:
            w0 = cpool.tile([128, n_out], f32, tag="w0")
            w1 = cpool.tile([128, n_out], f32, tag="w1")
            bt = cpool.tile([1, n_out], f32, tag="bt")
            st = cpool.tile([128, 1], f32, tag="st")
            ones = cpool.tile([1, 128], f32, tag="ones")
            nc.sync.dma_start(out=w0[:], in_=w[0:128, :])
            nc.sync.dma_start(out=w1[:], in_=w[128:256, :])
            nc.sync.dma_start(out=bt[:], in_=b[:])
            nc.sync.dma_start(out=st[:], in_=s[:])
            nc.vector.memset(ones[:], 1.0)
            for i in range(TILES):
                x0 = iop.tile([128, 128], f32, tag="x0")
                x1 = iop.tile([128, 128], f32, tag="x1")
                nc.sync.dma_start(out=x0[:], in_=xT[0:128, i * 128:(i + 1) * 128])
                nc.sync.dma_start(out=x1[:], in_=xT[128:256, i * 128:(i + 1) * 128])
                ps = psp.tile([128, n_out], f32, tag="ps", space="PSUM")
                nc.tensor.matmul(ps[:], lhsT=x0[:], rhs=w0[:], start=True, stop=False)
                nc.tensor.matmul(ps[:], lhsT=x1[:], rhs=w1[:], start=False, stop=False)
                nc.tensor.matmul(ps[:], lhsT=ones[:], rhs=bt[:], start=False, stop=True)
                t = iop.tile([128, n_out], f32, tag="t")
                o = iop.tile([128, n_out], f32, tag="o")
                nc.vector.tensor_scalar_mul(t[:], ps[:], st[:, :1])
                nc.vector.tensor_tensor(out=o[:], in0=ps[:], in1=t[:],
                                        op=mybir.AluOpType.max)
                nc.sync.dma_start(out=y[i * 128:(i + 1) * 128, :], in_=o[:])
    nc.compile()
    return nc


def _install_profile_shim():
    import sys, types
    try:
        import antenv
        from trn_agent_boot.trn_boot import _ntff_profile_via_ctypes
        mod = types.ModuleType("antenv.axon_hooks")
        hook = [_ntff_profile_via_ctypes('/opt/axon/libaxon_pjrt.so')]
        mod.get_axon_ntff_profile_hook = lambda: hook[0]
        mod.set_axon_ntff_profile_hook = lambda h: hook.__setitem__(0, h)
        sys.modules["antenv.axon_hooks"] = mod
        antenv.axon_hooks = mod
        return True
    except Exception:
        return False


_exec_ns = [0, 0]  # total ns, n launches profiled


def _gemm(x, w, bias, relu):
    """x [R<=ROWS_PAD, K<=256] @ w [K, N<=512] + bias, optional relu.
    Rows sharded across 8 cores on device. Returns [R, N] float32."""
    from concourse.bass_utils import run_bass_kernel_spmd

    R, K = x.shape
    Kw, N = w.shape
    n_out = 260 if N > 256 else 256
    key = ("nc", n_out)
    if key not in _cache:
        _cache[key] = _build_gemm(n_out)
    nc = _cache[key]
    assert K == Kw and K <= KDIM and N <= n_out and R <= ROWS_PAD
    xp = np.zeros((ROWS_PAD, KDIM), np.float32)
    xp[:R, :K] = x
    wp = np.zeros((KDIM, n_out), np.float32)
    wp[:K, :N] = w
    bp = np.zeros((1, n_out), np.float32)
    bp[0, :N] = bias
    sp = np.full((128, 1), 0.0 if relu else 1.0, np.float32)
    in_maps = []
    for c in range(N_CORES):
        sh = xp[c * ROWS_SH:(c + 1) * ROWS_SH]  # [ROWS_SH, 256]
        in_maps.append({"xT": np.ascontiguousarray(sh.T), "w": wp, "b": bp,
                        "s": sp})
    trace = bool(_cache.get("profile"))
    res = run_bass_kernel_spmd(nc, in_maps, list(range(N_CORES)), trace=trace)
    if trace and res.exec_time_ns:
        _exec_ns[0] += res.exec_time_ns
        _exec_ns[1] += 1
    out = np.concatenate([res.results[c]["y"] for c in range(N_CORES)], axis=0)
    return out[:R, :N]


def _prep_dir(src, dst, e12):
    """Sort one direction's edges by dst; return segment structure."""
    order = np.argsort(dst, kind="stable")
    dst_s = dst[order].astype(np.int64)
    src_s = src[order].astype(np.int64)
    e12_s = e12[order].astype(np.float32)
    if len(dst_s):
        starts = np.concatenate(([0], np.flatnonzero(np.diff(dst_s)) + 1))
        uniq = dst_s[starts]
        counts = np.diff(np.concatenate((starts, [len(dst_s)])))
    else:
        starts = np.zeros(0, np.int64)
        uniq = starts
        counts = starts
    return src_s, dst_s, e12_s, starts, uniq, counts


def _gat_host(hp, a_src, a_dst, bias, ed):
    """Aggregation for one direction given device-computed hp/a-vectors."""
    src_s, dst_s, e12_s, starts, uniq, counts = ed
    lg = a_src[src_s] + a_dst[dst_s] + e12_s
    lg = np.where(lg > 0, lg, np.float32(0.2) * lg).astype(np.float32)
    m = np.maximum.reduceat(lg, starts) if len(starts) else lg[:0]
    ex = np.exp(lg - np.repeat(m, counts))
    den = np.add.reduceat(ex, starts)
    alpha = ex / np.repeat(np.maximum(den, np.float32(1e-16)), counts)
    contrib = alpha[:, None].astype(np.float32) * hp[src_s]
    sums = np.add.reduceat(contrib, starts, axis=0)
    out = np.zeros((N_NODES, HALF), np.float32)
    out[uniq] = sums
    return out + bias[None, :].astype(np.float32)


def kernel(x, edge_index, edge_attr, batch_idx, params):
    if os.environ.get("BGNN_PROFILE"):
        _cache["profile"] = _install_profile_shim()
    x = np.asarray(x, np.float32)
    edge_index = np.asarray(edge_index)
    edge_attr = np.asarray(edge_attr, np.float32)
    batch_idx = np.asarray(batch_idx)
    p = {k: (np.asarray(v, np.float32) if not isinstance(v, (list, dict)) else v)
         for k, v in params.items()}
    layers = params["layers"]
    src_all, dst_all = (np.asarray(edge_index[0], np.int64),
                        np.asarray(edge_index[1], np.int64))
    fwd = ((edge_attr[:, 0] == 1) | (edge_attr[:, 2] == 1)
           | (edge_attr[:, 4] == 1))

    # fold (W_edge @ att_edge) for all 12 layer-directions -> e12 [E,12]
    C12 = np.stack([np.asarray(lp[d]["W_edge"], np.float32)
                    @ np.asarray(lp[d]["att_edge"], np.float32)
                    for lp in layers for d in ("fwd", "bwd")], axis=1)
    E12 = edge_attr @ C12  # [E, 12]

    eds = []
    for li in range(N_LAYERS):
        m = fwd
        eds.append((_prep_dir(src_all[m], dst_all[m], E12[m, 2 * li]),
                    _prep_dir(src_all[~m], dst_all[~m], E12[~m, 2 * li + 1])))

    # ---- embed (device) ----
    h = _gemm(x, np.asarray(p["embed_W"]), np.asarray(p["embed_b"]), relu=True)

    # ---- GAT layers ----
    for li, lp in enumerate(layers):
        Wf = np.asarray(lp["fwd"]["W"], np.float32)
        Wb = np.asarray(lp["bwd"]["W"], np.float32)
        cols = np.concatenate([
            Wf, Wb,
            (Wf @ np.asarray(lp["fwd"]["att_src"], np.float32))[:, None],
            (Wf @ np.asarray(lp["fwd"]["att_dst"], np.float32))[:, None],
            (Wb @ np.asarray(lp["bwd"]["att_src"], np.float32))[:, None],
            (Wb @ np.asarray(lp["bwd"]["att_dst"], np.float32))[:, None],
        ], axis=1)  # [256, 260]
        proj = _gemm(h, cols, np.zeros(260, np.float32), relu=False)
        hp_f, hp_b = proj[:, :HALF], proj[:, HALF:HID]
        asf, adf, asb, adb = (proj[:, 256], proj[:, 257], proj[:, 258],
                              proj[:, 259])
        f = _gat_host(hp_f, asf, adf, np.asarray(lp["fwd"]["bias"], np.float32),
                      eds[li][0])
        bb = _gat_host(hp_b, asb, adb, np.asarray(lp["bwd"]["bias"], np.float32),
                       eds[li][1])
        z = _gemm(np.concatenate([f, bb], axis=1),
                  np.asarray(lp["comb_W"], np.float32),
                  np.asarray(lp["comb_b"], np.float32), relu=False)
        mu = z.mean(axis=0, dtype=np.float32)
        var = z.var(axis=0, dtype=np.float32)
        zn = ((z - mu) / np.sqrt(var + np.float32(1e-5))
              * np.asarray(lp["bn_g"], np.float32)
              + np.asarray(lp["bn_b"], np.float32))
        h = np.maximum(zn, 0).astype(np.float32) + h

    # ---- pooling (host, tiny) ----
    bi = np.asarray(batch_idx, np.int64)
    cnt = np.bincount(bi, minlength=N_GRAPHS).astype(np.float32)
    sums = np.zeros((N_GRAPHS, HID), np.float32)
    np.add.at(sums, bi, h)
    meanp = sums / np.maximum(cnt, 1.0)[:, None]
    maxp = np.full((N_GRAPHS, HID), -np.inf, np.float32)
    np.maximum.at(maxp, bi, h)
    maxp = np.where(np.isfinite(maxp), maxp, 0.0).astype(np.float32)
    g = np.concatenate([meanp, maxp], axis=1) @ np.asarray(p["pool_W"])
    g = np.maximum(g + np.asarray(p["pool_b"]), 0).astype(np.float32)

    # ---- CNN head (host, tiny) ----
    z = g[:, None, :]  # [64, 1, 256]
    for cp in params["convs"]:
        W = np.asarray(cp["W"], np.float32)          # [oc, ic, 3]
        B, IC, L = z.shape
        zp = np.zeros((B, IC, L + 2), np.float32)
        zp[:, :, 1:L + 1] = z
        out = np.zeros((B, W.shape[0], L), np.float32)
        for k in range(3):
            out += np.einsum("oi,bil->bol", W[:, :, k], zp[:, :, k:k + L],
                             optimize=True)
        out += np.asarray(cp["b"], np.float32)[None, :, None]
        mu = out.mean(axis=(0, 2), dtype=np.float32)[None, :, None]
        var = out.var(axis=(0, 2), dtype=np.float32)[None, :, None]
        out = ((out - mu) / np.sqrt(var + np.float32(1e-5))
               * np.asarray(cp["g"], np.float32)[None, :, None]
               + np.asarray(cp["beta"], np.float32)[None, :, None])
        out = np.maximum(out, 0)
        z = np.maximum(out[:, :, ::2], out[:, :, 1::2]).astype(np.float32)
    z = z.reshape(z.shape[0], -1)
    fcs = params["fc"]
    for i, (W, b) in enumerate(fcs):
        z = z @ np.asarray(W, np.float32) + np.asarray(b, np.float32)
        if i < len(fcs) - 1:
            z = np.maximum(z, 0)
    return z.astype(np.float32)


def get_exec_time_ns():
    return _exec_ns[0]


# revision 6
# speedup vs baseline: 2.0630x; 2.0630x over previous
"""BGNN4VD (bidirectional GAT + CNN head) on 8 Trainium2 NeuronCores.

Strategy: the model's dense GEMMs (embed, 12 GAT projections + attention
vectors, 6 combine layers) are executed on-device via a single generic
row-sharded GEMM kernel (rows split 8 ways, weights replicated, N padded
to 512). The data-dependent edge softmax/aggregation and the tiny CNN
head run host-side between launches.
"""
import os
import numpy as np

N_NODES = 100000
N_GRAPHS = 64
HID = 256
HALF = 128
N_LAYERS = 6
N_CORES = 8
ROWS_PAD = 100352          # 8 * 12544, multiple of 128*8
ROWS_SH = ROWS_PAD // N_CORES   # 12544 rows per core
KDIM = 256                 # padded contraction dim
NDIM = 512                 # padded output dim
TILES = ROWS_SH // 128     # 98

_cache = {}


def _build_gemm(n_out):
    """y[12544,n_out] = blend(xT.T @ W + b) per core; blend = max(z, z*s)."""
    from concourse import mybir, bacc
    import concourse.tile as tile

    nc = bacc.Bacc("TRN2", target_bir_lowering=False, debug=False,
                   num_devices=N_CORES)
    f32 = mybir.dt.float32
    f32r = mybir.dt.float32r
    xT = nc.dram_tensor("xT", [KDIM, ROWS_SH], f32r, kind="ExternalInput")
    w = nc.dram_tensor("w", [KDIM, n_out], f32r, kind="ExternalInput")
    b = nc.dram_tensor("b", [1, n_out], f32r, kind="ExternalInput")
    s = nc.dram_tensor("s", [128, 1], f32, kind="ExternalInput")
    onesd = nc.dram_tensor("onesd", [1, 128], f32r, kind="ExternalInput")
    y = nc.dram_tensor("y", [ROWS_SH, n_out], f32, kind="ExternalOutput")

    with tile.TileContext(nc) as tc:
        with tc.tile_pool(name="const", bufs=1) as cpool, \
             tc.tile_pool(name="io", bufs=4) as iop, \
             tc.tile_pool(name="ps", bufs=4, space="PSUM") as psp:
            w0 = cpool.tile([128, n_out], f32r, tag="w0")
            w1 = cpool.tile([128, n_out], f32r, tag="w1")
            bt = cpool.tile([1, n_out], f32r, tag="bt")
            st = cpool.tile([128, 1], f32, tag="st")
            ones = cpool.tile([1, 128], f32r, tag="ones")
            nc.sync.dma_start(out=w0[:], in_=w[0:128, :])
            nc.sync.dma_start(out=w1[:], in_=w[128:256, :])
            nc.sync.dma_start(out=bt[:], in_=b[:])
            nc.sync.dma_start(out=st[:], in_=s[:])
            nc.sync.dma_start(out=ones[:], in_=onesd[:])
            GRP = 4  # node tiles per input DMA (512 contiguous cols)
            for g in range(TILES // GRP + (1 if TILES % GRP else 0)):
                i0 = g * GRP
                nt = min(GRP, TILES - i0)
                x0 = iop.tile([128, GRP * 128], f32r, tag="x0")
                x1 = iop.tile([128, GRP * 128], f32r, tag="x1")
                c0, c1 = i0 * 128, (i0 + nt) * 128
                nc.sync.dma_start(out=x0[:, :nt * 128], in_=xT[0:128, c0:c1])
                nc.sync.dma_start(out=x1[:, :nt * 128], in_=xT[128:256, c0:c1])
                for j in range(nt):
                    i = i0 + j
                    ps = psp.tile([128, n_out], f32, tag="ps", space="PSUM")
                    sl = slice(j * 128, (j + 1) * 128)
                    nc.tensor.matmul(ps[:], lhsT=x0[:, sl], rhs=w0[:],
                                     start=True, stop=False)
                    nc.tensor.matmul(ps[:], lhsT=x1[:, sl], rhs=w1[:],
                                     start=False, stop=False)
                    nc.tensor.matmul(ps[:], lhsT=ones[:], rhs=bt[:],
                                     start=False, stop=True)
                    t = iop.tile([128, n_out], f32, tag="t")
                    o = iop.tile([128, n_out], f32, tag="o")
                    nc.vector.tensor_scalar_mul(t[:], ps[:], st[:, :1])
                    nc.vector.tensor_tensor(out=o[:], in0=ps[:], in1=t[:],
                                            op=mybir.AluOpType.max)
                    nc.sync.dma_start(out=y[i * 128:(i + 1) * 128, :], in_=o[:])
    nc.compile()
    return nc


def _install_profile_shim():
    import sys, types
    try:
        import antenv
        from trn_agent_boot.trn_boot import _ntff_profile_via_ctypes
        mod = types.ModuleType("antenv.axon_hooks")
        hook = [_ntff_profile_via_ctypes('/opt/axon/libaxon_pjrt.so')]
        mod.get_axon_ntff_profile_hook = lambda: hook[0]
        mod.set_axon_ntff_profile_hook = lambda h: hook.__setitem__(0, h)
        sys.modules["antenv.axon_hooks"] = mod
        antenv.axon_hooks = mod
        return True
    except Exception:
        return False


_exec_ns = [0, 0]  # total ns, n launches profiled


def _gemm(x, w, bias, relu):
    """x [R<=ROWS_PAD, K<=256] @ w [K, N<=512] + bias, optional relu.
    Rows sharded across 8 cores on device. Returns [R, N] float32."""
    from concourse.bass_utils import run_bass_kernel_spmd

    R, K = x.shape
    Kw, N = w.shape
    n_out = 260 if N > 256 else 256
    key = ("nc", n_out)
    if key not in _cache:
        _cache[key] = _build_gemm(n_out)
    nc = _cache[key]
    assert K == Kw and K <= KDIM and N <= n_out and R <= ROWS_PAD
    xp = np.zeros((ROWS_PAD, KDIM), np.float32)
    xp[:R, :K] = x
    wp = np.zeros((KDIM, n_out), np.float32)
    wp[:K, :N] = w
    bp = np.zeros((1, n_out), np.float32)
    bp[0, :N] = bias
    sp = np.full((128, 1), 0.0 if relu else 1.0, np.float32)
    in_maps = []
    for c in range(N_CORES):
        sh = xp[c * ROWS_SH:(c + 1) * ROWS_SH]  # [ROWS_SH, 256]
        in_maps.append({"xT": np.ascontiguousarray(sh.T), "w": wp, "b": bp,
                        "s": sp, "onesd": np.ones((1, 128), np.float32)})
    trace = bool(_cache.get("profile"))
    res = run_bass_kernel_spmd(nc, in_maps, list(range(N_CORES)), trace=trace)
    if trace and res.exec_time_ns:
        _exec_ns[0] += res.exec_time_ns
        _exec_ns[1] += 1
    out = np.concatenate([res.results[c]["y"] for c in range(N_CORES)], axis=0)
    return out[:R, :N]


def _prep_dir(src, dst, e12):
    """Sort one direction's edges by dst; return segment structure."""
    order = np.argsort(dst, kind="stable")
    dst_s = dst[order].astype(np.int64)
    src_s = src[order].astype(np.int64)
    e12_s = e12[order].astype(np.float32)
    if len(dst_s):
        starts = np.concatenate(([0], np.flatnonzero(np.diff(dst_s)) + 1))
        uniq = dst_s[starts]
        counts = np.diff(np.concatenate((starts, [len(dst_s)])))
    else:
        starts = np.zeros(0, np.int64)
        uniq = starts
        counts = starts
    return src_s, dst_s, e12_s, starts, uniq, counts


def _gat_host(hp, a_src, a_dst, bias, ed):
    """Aggregation for one direction given device-computed hp/a-vectors."""
    src_s, dst_s, e12_s, starts, uniq, counts = ed
    lg = a_src[src_s] + a_dst[dst_s] + e12_s
    lg = np.where(lg > 0, lg, np.float32(0.2) * lg).astype(np.float32)
    m = np.maximum.reduceat(lg, starts) if len(starts) else lg[:0]
    ex = np.exp(lg - np.repeat(m, counts))
    den = np.add.reduceat(ex, starts)
    alpha = ex / np.repeat(np.maximum(den, np.float32(1e-16)), counts)
    contrib = alpha[:, None].astype(np.float32) * hp[src_s]
    sums = np.add.reduceat(contrib, starts, axis=0)
    out = np.zeros((N_NODES, HALF), np.float32)
    out[uniq] = sums
    return out + bias[None, :].astype(np.float32)


def kernel(x, edge_index, edge_attr, batch_idx, params):
    if os.environ.get("BGNN_PROFILE"):
        _cache["profile"] = _install_profile_shim()
    x = np.asarray(x, np.float32)
    edge_index = np.asarray(edge_index)
    edge_attr = np.asarray(edge_attr, np.float32)
    batch_idx = np.asarray(batch_idx)
    p = {k: (np.asarray(v, np.float32) if not isinstance(v, (list, dict)) else v)
         for k, v in params.items()}
    layers = params["layers"]
    src_all, dst_all = (np.asarray(edge_index[0], np.int64),
                        np.asarray(edge_index[1], np.int64))
    fwd = ((edge_attr[:, 0] == 1) | (edge_attr[:, 2] == 1)
           | (edge_attr[:, 4] == 1))

    # fold (W_edge @ att_edge) for all 12 layer-directions -> e12 [E,12]
    C12 = np.stack([np.asarray(lp[d]["W_edge"], np.float32)
                    @ np.asarray(lp[d]["att_edge"], np.float32)
                    for lp in layers for d in ("fwd", "bwd")], axis=1)
    E12 = edge_attr @ C12  # [E, 12]

    eds = []
    for li in range(N_LAYERS):
        m = fwd
        eds.append((_prep_dir(src_all[m], dst_all[m], E12[m, 2 * li]),
                    _prep_dir(src_all[~m], dst_all[~m], E12[~m, 2 * li + 1])))

    # ---- embed (device) ----
    h = _gemm(x, np.asarray(p["embed_W"]), np.asarray(p["embed_b"]), relu=True)

    # ---- GAT layers ----
    for li, lp in enumerate(layers):
        Wf = np.asarray(lp["fwd"]["W"], np.float32)
        Wb = np.asarray(lp["bwd"]["W"], np.float32)
        cols = np.concatenate([
            Wf, Wb,
            (Wf @ np.asarray(lp["fwd"]["att_src"], np.float32))[:, None],
            (Wf @ np.asarray(lp["fwd"]["att_dst"], np.float32))[:, None],
            (Wb @ np.asarray(lp["bwd"]["att_src"], np.float32))[:, None],
            (Wb @ np.asarray(lp["bwd"]["att_dst"], np.float32))[:, None],
        ], axis=1)  # [256, 260]
        proj = _gemm(h, cols, np.zeros(260, np.float32), relu=False)
        hp_f, hp_b = proj[:, :HALF], proj[:, HALF:HID]
        asf, adf, asb, adb = (proj[:, 256], proj[:, 257], proj[:, 258],
                              proj[:, 259])
        f = _gat_host(hp_f, asf, adf, np.asarray(lp["fwd"]["bias"], np.float32),
                      eds[li][0])
        bb = _gat_host(hp_b, asb, adb, np.asarray(lp["bwd"]["bias"], np.float32),
                       eds[li][1])
        z = _gemm(np.concatenate([f, bb], axis=1),
                  np.asarray(lp["comb_W"], np.float32),
                  np.asarray(lp["comb_b"], np.float32), relu=False)
        mu = z.mean(axis=0, dtype=np.float32)
        var = z.var(axis=0, dtype=np.float32)
        zn = ((z - mu) / np.sqrt(var + np.float32(1e-5))
              * np.asarray(lp["bn_g"], np.float32)
              + np.asarray(lp["bn_b"], np.float32))
        h = np.maximum(zn, 0).astype(np.float32) + h

    # ---- pooling (host, tiny) ----
    bi = np.asarray(batch_idx, np.int64)
    cnt = np.bincount(bi, minlength=N_GRAPHS).astype(np.float32)
    sums = np.zeros((N_GRAPHS, HID), np.float32)
    np.add.at(sums, bi, h)
    meanp = sums / np.maximum(cnt, 1.0)[:, None]
    maxp = np.full((N_GRAPHS, HID), -np.inf, np.float32)
    np.maximum.at(maxp, bi, h)
    maxp = np.where(np.isfinite(maxp), maxp, 0.0).astype(np.float32)
    g = np.concatenate([meanp, maxp], axis=1) @ np.asarray(p["pool_W"])
    g = np.maximum(g + np.asarray(p["pool_b"]), 0).astype(np.float32)

    # ---- CNN head (host, tiny) ----
    z = g[:, None, :]  # [64, 1, 256]
    for cp in params["convs"]:
        W = np.asarray(cp["W"], np.float32)          # [oc, ic, 3]
        B, IC, L = z.shape
        zp = np.zeros((B, IC, L + 2), np.float32)
        zp[:, :, 1:L + 1] = z
        out = np.zeros((B, W.shape[0], L), np.float32)
        for k in range(3):
            out += np.einsum("oi,bil->bol", W[:, :, k], zp[:, :, k:k + L],
                             optimize=True)
        out += np.asarray(cp["b"], np.float32)[None, :, None]
        mu = out.mean(axis=(0, 2), dtype=np.float32)[None, :, None]
        var = out.var(axis=(0, 2), dtype=np.float32)[None, :, None]
        out = ((out - mu) / np.sqrt(var + np.float32(1e-5))
               * np.asarray(cp["g"], np.float32)[None, :, None]
               + np.asarray(cp["beta"], np.float32)[None, :, None])
        out = np.maximum(out, 0)
        z = np.maximum(out[:, :, ::2], out[:, :, 1::2]).astype(np.float32)
    z = z.reshape(z.shape[0], -1)
    fcs = params["fc"]
    for i, (W, b) in enumerate(fcs):
        z = z @ np.asarray(W, np.float32) + np.asarray(b, np.float32)
        if i < len(fcs) - 1:
            z = np.maximum(z, 0)
    return z.astype(np.float32)


def get_exec_time_ns():
    return _exec_ns[0]
